# revision 10
# baseline (speedup 1.0000x reference)
"""MoE FFN (8 experts, top-2) on 8 Trainium2 NeuronCores.

Expert parallelism: the (tiny) router runs on host with the exact same jax
ops as the reference; tokens are dispatched to their top-2 experts; core e
runs expert e's FFN over its routed tokens (capacity-padded so all cores run
the same SPMD program); the host applies the combine weights and
scatter-adds the two expert outputs per token.

On-device layout: all matmul operands keep the contraction dim on SBUF
partitions. Weights are host-pre-transposed (w1t = w1[e].T contiguous,
w2t = w2[e].T contiguous, bf16) and stay resident in SBUF; activations live
as G.T = gelu(X W1.T).T in [H, tokens] layout so layer 2 consumes them
directly. PSUM accumulates in f32; biases are applied by ScalarE.
"""

import numpy as np
import ml_dtypes

N_EXPERTS = 8
TOP_K = 2
C = 1024
H = 4096
P = 128
T_TILE = 512
KO1 = C // P   # 8 contraction chunks for layer 1
KO2 = H // P   # 32 contraction chunks for layer 2

_nc_cache = {}


def _build_nc(cap: int, act: str = "gelu"):
    import concourse.mybir as mybir
    import concourse.tile as tile
    from concourse import bacc

    bf16 = mybir.dt.bfloat16
    f32 = mybir.dt.float32

    nc = bacc.Bacc()
    xt = nc.dram_tensor("xt", [C, cap], bf16, kind="ExternalInput")
    w1t = nc.dram_tensor("w1t", [C, H], bf16, kind="ExternalInput")
    w2t = nc.dram_tensor("w2t", [H, C], bf16, kind="ExternalInput")
    # biases come host-pre-swizzled: [P, H//P] / [P, C//P], partition-major
    b1 = nc.dram_tensor("b1", [P, KO2], f32, kind="ExternalInput")
    b2 = nc.dram_tensor("b2", [P, KO1], f32, kind="ExternalInput")
    yt = nc.dram_tensor("yt", [C, cap], f32, kind="ExternalOutput")

    xt_r = xt.rearrange("(ko ki) t -> ki ko t", ki=P)
    w1t_r = w1t.rearrange("(ko ki) h -> ki ko h", ki=P)
    w2t_r = w2t.rearrange("(ko ki) c -> ki ko c", ki=P)
    yt_r = yt.rearrange("(co p) t -> p co t", p=P)

    n_full, rem = divmod(cap, T_TILE)
    tiles = [T_TILE] * n_full + ([rem] if rem else [])

    gelu = {
        "gelu": mybir.ActivationFunctionType.Gelu_apprx_tanh,
        "gelu_erf": mybir.ActivationFunctionType.Gelu,
        "tanh": mybir.ActivationFunctionType.Tanh,
    }[act]
    ident = mybir.ActivationFunctionType.Identity

    with tile.TileContext(nc) as tc:
        with (
            tc.tile_pool(name="const", bufs=1) as const,
            tc.tile_pool(name="xp", bufs=2) as xp,
            tc.tile_pool(name="gp", bufs=1) as gp,
            tc.tile_pool(name="yp", bufs=4) as yp,
            tc.tile_pool(name="psum", bufs=6, space="PSUM") as psum,
        ):
            w1_sb = const.tile([P, KO1, H], bf16, tag="w1")
            w2_sb = const.tile([P, KO2, C], bf16, tag="w2")
            b1_st = const.tile([P, KO2], f32, tag="b1s")
            b2_st = const.tile([P, KO1], f32, tag="b2s")
            b1_sb = const.tile([P, KO2], f32, tag="b1")
            b2_sb = const.tile([P, KO1], f32, tag="b2")
            # split weight loads so they spread across DMA queues
            for ko in range(KO1):
                nc.sync.dma_start(w1_sb[:, ko : ko + 1, :], w1t_r[:, ko : ko + 1, :])
            for ko in range(KO2):
                nc.sync.dma_start(w2_sb[:, ko : ko + 1, :], w2t_r[:, ko : ko + 1, :])
            nc.sync.dma_start(b1_st[:], b1[:])
            nc.sync.dma_start(b2_st[:], b2[:])
            # stage through DVE so ScalarE never waits on raw DMA queue sems
            nc.vector.tensor_copy(b1_sb[:], b1_st[:])
            nc.vector.tensor_copy(b2_sb[:], b2_st[:])

            t0 = 0
            for T in tiles:
                x_sb = xp.tile([P, KO1, T_TILE], bf16, tag="x")
                nc.sync.dma_start(x_sb[:, :, :T], xt_r[:, :, t0 : t0 + T])
                g_sb = gp.tile([P, KO2, T_TILE], bf16, tag="g")
                for m in range(KO2):
                    ph = psum.tile([P, T_TILE], f32, tag="ps")
                    for ko in range(KO1):
                        nc.tensor.matmul(
                            ph[:, :T],
                            w1_sb[:, ko, m * P : (m + 1) * P],
                            x_sb[:, ko, :T],
                            start=(ko == 0),
                            stop=(ko == KO1 - 1),
                        )
                    nc.scalar.activation(
                        g_sb[:, m, :T], ph[:, :T], gelu, bias=b1_sb[:, m : m + 1]
                    )
                for co in range(KO1):
                    py = psum.tile([P, T_TILE], f32, tag="ps")
                    for ho in range(KO2):
                        nc.tensor.matmul(
                            py[:, :T],
                            w2_sb[:, ho, co * P : (co + 1) * P],
                            g_sb[:, ho, :T],
                            start=(ho == 0),
                            stop=(ho == KO2 - 1),
                        )
                    y_sb = yp.tile([P, T_TILE], f32, tag="y")
                    nc.scalar.activation(
                        y_sb[:, :T], py[:, :T], ident, bias=b2_sb[:, co : co + 1]
                    )
                    nc.sync.dma_start(yt_r[:, co, t0 : t0 + T], y_sb[:, :T])
                t0 += T
    nc.finalize()
    return nc


def _route(flat_f32: np.ndarray, gate_w: np.ndarray):
    """Router, bit-matching the reference's jax ops (same env/backend)."""
    import jax
    import jax.numpy as jnp

    logits = jnp.asarray(flat_f32) @ jnp.asarray(gate_w).T
    probs = jax.nn.softmax(logits, axis=-1)
    top_p, top_i = jax.lax.top_k(probs, TOP_K)
    weights = top_p / (jnp.sum(top_p, axis=-1, keepdims=True) + 1e-8)
    return np.asarray(top_i), np.asarray(weights)


# results of the last device run, for test harness introspection
last_result = None


def _ensure_ntff_hook():
    """bass_utils' trace path imports antenv.axon_hooks, which the agent
    image's antenv lacks. Build the hook from trn_agent_boot's ctypes
    shim and inject a stand-in module."""
    import sys
    import types

    if "antenv.axon_hooks" in sys.modules:
        return
    try:
        from trn_agent_boot.trn_boot import _ntff_profile_via_ctypes

        hook = _ntff_profile_via_ctypes("/opt/axon/libaxon_pjrt.so")
    except Exception:
        hook = None
    m = types.ModuleType("antenv.axon_hooks")
    m.get_axon_ntff_profile_hook = lambda: hook
    m.set_axon_ntff_profile_hook = lambda h: None
    sys.modules["antenv.axon_hooks"] = m


def kernel(x, gate_w, w1, b1, w2, b2):
    from concourse.bass_utils import run_bass_kernel_spmd

    x = np.asarray(x)
    B, N, _ = x.shape
    flat = np.ascontiguousarray(x.reshape(-1, C), dtype=np.float32)
    T = flat.shape[0]

    top_i, weights = _route(flat, np.asarray(gate_w, dtype=np.float32))

    # token ids and combine weights per expert
    idx_e = []
    g_e = []
    for e in range(N_EXPERTS):
        rows, cols = np.nonzero(top_i == e)
        idx_e.append(rows.astype(np.int64))
        g_e.append(weights[rows, cols].astype(np.float32))
    counts = np.array([len(i) for i in idx_e])
    cap = max(int(-(-counts.max() // P) * P), P)

    nc = _nc_cache.get(cap)
    if nc is None:
        nc = _build_nc(cap)
        _nc_cache[cap] = nc

    bf16 = ml_dtypes.bfloat16
    w1_t = np.ascontiguousarray(np.asarray(w1).transpose(0, 2, 1)).astype(bf16)
    w2_t = np.ascontiguousarray(np.asarray(w2).transpose(0, 2, 1)).astype(bf16)
    # pre-swizzle biases to [P, n_chunks]: partition p of chunk m holds b[m*P+p]
    b1_f = np.ascontiguousarray(
        np.asarray(b1, dtype=np.float32).reshape(N_EXPERTS, KO2, P).transpose(0, 2, 1)
    )
    b2_f = np.ascontiguousarray(
        np.asarray(b2, dtype=np.float32).reshape(N_EXPERTS, KO1, P).transpose(0, 2, 1)
    )

    in_maps = []
    for e in range(N_EXPERTS):
        xe = np.zeros((C, cap), dtype=bf16)
        xe[:, : counts[e]] = flat[idx_e[e]].T.astype(bf16)
        in_maps.append(
            {
                "xt": xe,
                "w1t": w1_t[e],
                "w2t": w2_t[e],
                "b1": b1_f[e],
                "b2": b2_f[e],
            }
        )

    import os

    trace = bool(int(os.environ.get("MOE_TRACE", "0")))
    if trace:
        _ensure_ntff_hook()

    global last_result
    res = run_bass_kernel_spmd(
        nc,
        in_maps,
        core_ids=list(range(N_EXPERTS)),
        trace=trace,
    )
    last_result = res

    out = np.zeros((T, C), dtype=np.float32)
    for e in range(N_EXPERTS):
        ye = res.results[e]["yt"]  # [C, cap] f32
        cnt = counts[e]
        out[idx_e[e]] += g_e[e][:, None] * ye[:, :cnt].T
    return out.reshape(B, N, C)


# revision 24
# speedup vs baseline: 1.0846x; 1.0846x over previous
"""MoE FFN (8 experts, top-2) on 8 Trainium2 NeuronCores.

Expert parallelism: the (tiny) router runs on host with the exact same jax
ops as the reference; tokens are dispatched to their top-2 experts; core e
runs expert e's FFN over its routed tokens (capacity-padded so all cores run
the same SPMD program); the host applies the combine weights and
scatter-adds the two expert outputs per token.

On-device layout: all matmul operands keep the contraction dim on SBUF
partitions. Weights are host-pre-transposed (w1t = w1[e].T contiguous,
w2t = w2[e].T contiguous, bf16) and stay resident in SBUF; activations live
as G.T = gelu(X W1.T).T in [H, tokens] layout so layer 2 consumes them
directly. PSUM accumulates in f32; layer-1 bias rides the gelu on ScalarE,
layer-2 bias is fused into the PSUM eviction on VectorE.
"""

import numpy as np
import ml_dtypes

N_EXPERTS = 8
TOP_K = 2
C = 1024
H = 4096
P = 128
T_TILE = 512
KO1 = C // P   # 8 contraction chunks for layer 1
KO2 = H // P   # 32 contraction chunks for layer 2

_nc_cache = {}


def _build_nc(cap: int, act: str = "gelu"):
    import concourse.mybir as mybir
    import concourse.tile as tile
    from concourse import bacc

    bf16 = mybir.dt.bfloat16
    f32 = mybir.dt.float32

    nc = bacc.Bacc()
    xt = nc.dram_tensor("xt", [C, cap], bf16, kind="ExternalInput")
    w1t = nc.dram_tensor("w1t", [C, H], bf16, kind="ExternalInput")
    w2t = nc.dram_tensor("w2t", [H, C], bf16, kind="ExternalInput")
    # biases come host-pre-swizzled: [P, H//P] / [P, C//P], partition-major
    b1 = nc.dram_tensor("b1", [P, KO2], f32, kind="ExternalInput")
    b2 = nc.dram_tensor("b2", [P, KO1], f32, kind="ExternalInput")
    yt = nc.dram_tensor("yt", [C, cap], f32, kind="ExternalOutput")

    xt_r = xt.rearrange("(ko ki) t -> ki ko t", ki=P)
    w1t_r = w1t.rearrange("(ko ki) h -> ki ko h", ki=P)
    w2t_r = w2t.rearrange("(ko ki) c -> ki ko c", ki=P)
    yt_r = yt.rearrange("(co p) t -> p co t", p=P)

    # Balanced token tiles: per-tile matmul count is fixed (512) whatever T
    # is, so equal splits amortize issue overhead best; tiles under ~256
    # tokens fall below the LDWEIGHTS floor (~100ns/MM) and waste PE time.
    n_tiles = -(-cap // T_TILE)
    k = cap // P
    tiles = [(k // n_tiles + (1 if i < k % n_tiles else 0)) * P for i in range(n_tiles)]
    assert sum(tiles) == cap and all(t <= T_TILE for t in tiles)

    gelu = {
        "gelu": mybir.ActivationFunctionType.Gelu_apprx_tanh,
        "gelu_erf": mybir.ActivationFunctionType.Gelu,
        "tanh": mybir.ActivationFunctionType.Tanh,
    }[act]

    with tile.TileContext(nc) as tc:
        with (
            tc.tile_pool(name="const", bufs=1) as const,
            tc.tile_pool(name="xp", bufs=2) as xp,
            tc.tile_pool(name="gp", bufs=1) as gp,
            tc.tile_pool(name="yp", bufs=4) as yp,
            tc.tile_pool(name="psum", bufs=8, space="PSUM") as psum,
        ):
            w1_sb = const.tile([P, KO1, H], bf16, tag="w1")
            w2_sb = const.tile([P, KO2, C], bf16, tag="w2")
            b1_sb = const.tile([P, KO2], f32, tag="b1")
            b2_sb = const.tile([P, KO1], f32, tag="b2")
            # The HWDGE stream drains serially in program order, so issue
            # loads in exactly the order the first layer-1 m-tiles consume
            # them: x(t0) ko-chunks interleaved with the first w1 h-quarter,
            # then biases, the rest of w1, then w2. Anything queued behind
            # the 16MB of weights would stall the first matmuls ~45us.
            x_tiles = {}
            x_tiles[0] = xp.tile([P, KO1, T_TILE], bf16, tag="x", name="x0")
            QW = 1024
            for ko in range(KO1):
                nc.sync.dma_start(
                    x_tiles[0][:, ko : ko + 1, : tiles[0]],
                    xt_r[:, ko : ko + 1, : tiles[0]],
                )
                nc.sync.dma_start(
                    w1_sb[:, ko : ko + 1, 0:QW], w1t_r[:, ko : ko + 1, 0:QW]
                )
            nc.sync.dma_start(b1_sb[:], b1[:])
            nc.sync.dma_start(b2_sb[:], b2[:])
            for q in range(QW, H, QW):
                for ko in range(KO1):
                    nc.sync.dma_start(
                        w1_sb[:, ko : ko + 1, q : q + QW],
                        w1t_r[:, ko : ko + 1, q : q + QW],
                    )
            for ko in range(KO2):
                nc.sync.dma_start(w2_sb[:, ko : ko + 1, :], w2t_r[:, ko : ko + 1, :])

            t0 = 0
            for ti, T in enumerate(tiles):
                # prefetch next x tile ahead of this tile's output DMAs
                if ti + 1 < len(tiles):
                    nt = tiles[ti + 1]
                    nt0 = t0 + T
                    x_tiles[ti + 1] = xp.tile(
                        [P, KO1, T_TILE], bf16, tag="x", name=f"x{ti + 1}"
                    )
                    nc.sync.dma_start(
                        x_tiles[ti + 1][:, :, :nt], xt_r[:, :, nt0 : nt0 + nt]
                    )
                x_sb = x_tiles.pop(ti)
                g_sb = gp.tile([P, KO2, T_TILE], bf16, tag="g")
                for m in range(KO2):
                    ph = psum.tile([P, T_TILE], f32, tag="ps")
                    for ko in range(KO1):
                        nc.tensor.matmul(
                            ph[:, :T],
                            w1_sb[:, ko, m * P : (m + 1) * P],
                            x_sb[:, ko, :T],
                            start=(ko == 0),
                            stop=(ko == KO1 - 1),
                        )
                    nc.scalar.activation(
                        g_sb[:, m, :T], ph[:, :T], gelu, bias=b1_sb[:, m : m + 1]
                    )
                for co in range(KO1):
                    py = psum.tile([P, T_TILE], f32, tag="ps")
                    for ho in range(KO2):
                        nc.tensor.matmul(
                            py[:, :T],
                            w2_sb[:, ho, co * P : (co + 1) * P],
                            g_sb[:, ho, :T],
                            start=(ho == 0),
                            stop=(ho == KO2 - 1),
                        )
                    y_sb = yp.tile([P, T_TILE], f32, tag="y")
                    nc.vector.tensor_scalar_add(
                        y_sb[:, :T], py[:, :T], b2_sb[:, co : co + 1]
                    )
                    nc.sync.dma_start(yt_r[:, co, t0 : t0 + T], y_sb[:, :T])
                t0 += T
    nc.finalize()
    return nc


def _route(flat_f32: np.ndarray, gate_w: np.ndarray):
    """Router, bit-matching the reference's jax ops (same env/backend)."""
    import jax
    import jax.numpy as jnp

    logits = jnp.asarray(flat_f32) @ jnp.asarray(gate_w).T
    probs = jax.nn.softmax(logits, axis=-1)
    top_p, top_i = jax.lax.top_k(probs, TOP_K)
    weights = top_p / (jnp.sum(top_p, axis=-1, keepdims=True) + 1e-8)
    return np.asarray(top_i), np.asarray(weights)


# results of the last device run, for test harness introspection
last_result = None


def _ensure_ntff_hook():
    """bass_utils' trace path imports antenv.axon_hooks, which the agent
    image's antenv lacks. Build the hook from trn_agent_boot's ctypes
    shim and inject a stand-in module."""
    import sys
    import types

    if "antenv.axon_hooks" in sys.modules:
        return
    try:
        from trn_agent_boot.trn_boot import _ntff_profile_via_ctypes

        hook = _ntff_profile_via_ctypes("/opt/axon/libaxon_pjrt.so")
    except Exception:
        hook = None
    m = types.ModuleType("antenv.axon_hooks")
    m.get_axon_ntff_profile_hook = lambda: hook
    m.set_axon_ntff_profile_hook = lambda h: None
    sys.modules["antenv.axon_hooks"] = m


def kernel(x, gate_w, w1, b1, w2, b2):
    from concourse.bass_utils import run_bass_kernel_spmd

    x = np.asarray(x)
    B, N, _ = x.shape
    flat = np.ascontiguousarray(x.reshape(-1, C), dtype=np.float32)
    T = flat.shape[0]

    top_i, weights = _route(flat, np.asarray(gate_w, dtype=np.float32))

    # token ids and combine weights per expert
    idx_e = []
    g_e = []
    for e in range(N_EXPERTS):
        rows, cols = np.nonzero(top_i == e)
        idx_e.append(rows.astype(np.int64))
        g_e.append(weights[rows, cols].astype(np.float32))
    counts = np.array([len(i) for i in idx_e])
    cap = max(int(-(-counts.max() // P) * P), P)

    nc = _nc_cache.get(cap)
    if nc is None:
        nc = _build_nc(cap)
        _nc_cache[cap] = nc

    bf16 = ml_dtypes.bfloat16
    w1_t = np.ascontiguousarray(np.asarray(w1).transpose(0, 2, 1)).astype(bf16)
    w2_t = np.ascontiguousarray(np.asarray(w2).transpose(0, 2, 1)).astype(bf16)
    # pre-swizzle biases to [P, n_chunks]: partition p of chunk m holds b[m*P+p]
    b1_f = np.ascontiguousarray(
        np.asarray(b1, dtype=np.float32).reshape(N_EXPERTS, KO2, P).transpose(0, 2, 1)
    )
    b2_f = np.ascontiguousarray(
        np.asarray(b2, dtype=np.float32).reshape(N_EXPERTS, KO1, P).transpose(0, 2, 1)
    )

    in_maps = []
    for e in range(N_EXPERTS):
        xe = np.zeros((C, cap), dtype=bf16)
        xe[:, : counts[e]] = flat[idx_e[e]].T.astype(bf16)
        in_maps.append(
            {
                "xt": xe,
                "w1t": w1_t[e],
                "w2t": w2_t[e],
                "b1": b1_f[e],
                "b2": b2_f[e],
            }
        )

    import os

    trace = bool(int(os.environ.get("MOE_TRACE", "0")))
    if trace:
        _ensure_ntff_hook()

    global last_result
    res = run_bass_kernel_spmd(
        nc,
        in_maps,
        core_ids=list(range(N_EXPERTS)),
        trace=trace,
    )
    last_result = res

    out = np.zeros((T, C), dtype=np.float32)
    for e in range(N_EXPERTS):
        ye = res.results[e]["yt"]  # [C, cap] f32
        cnt = counts[e]
        out[idx_e[e]] += g_e[e][:, None] * ye[:, :cnt].T
    return out.reshape(B, N, C)
